# revision 31
# baseline (speedup 1.0000x reference)
"""Trainium2 Bass kernel for windowed (block-diagonal) multi-head video attention.

Problem: x:[2,8192,1024] -> qkv proj -> 3D-window (2,8,8) attention over a
(8,32,32) token grid, 16 heads x 64 dim -> out proj -> [2,8192,1024].

Sharding: 8 cores, data-parallel over (batch, t-window-group); the slab
x[b, it*2048:(it+1)*2048, :] holds the 16 independent (h,w)-windows with
t in {2it, 2it+1}.

This version runs the whole datapath in bf16 on the PE (1 cyc/col at any
ap size, vs f32r's 4 cyc/col below ap=256, so the ap=128 attention matmuls
are 4x faster) and moves all layout work to the host:
  - x is pre-transposed and window-gathered on the host into
    [group, 128 c-part, KC, 512 tok] so the kernel needs no PE transposes
    and no strided gather DMAs; qkv/proj weights are pre-chunked so each
    oc-chunk is one contiguous DMA that arrives in consumption order.
  - windows processed in groups of GW=4 (tok dim 512 = full PSUM bank).
  - attention: S^T = K_h Q_h^T per head (bf16), exp on ACT, A.V matmul with
    a per-head ones column producing the softmax denominator for free; the
    16 head-blocks of a window share one 4-bank PSUM tile and the recip
    (Ln + exp(-x) on ACT) runs per half-window as soon as each 8-head
    denominator stretch lands; 1/den is partition-broadcast via a K=1
    matmul and applied on DVE (R copies alternate ACT/DVE).
  - window finish (recip-broadcast, normalize, out-proj) is software-
    pipelined one window behind S/AV: all four normalize chains are emitted
    at the top of the next window (their recips completed mid-window), the
    V/S block covers them, and the out-proj interleaves with the next A.V
    so the PE never waits on the ACT/DVE chains.
Output is written window-major and un-permuted on the host.
"""

import sys

for _p in ("/opt/trn_rl_repo",):
    if _p not in sys.path:
        sys.path.insert(0, _p)

import numpy as np
import ml_dtypes

B, T, H, W = 2, 8, 32, 32
C, NH, HD = 1024, 16, 64
WT, WH, WW = 2, 8, 8
N = T * H * W              # 8192 tokens
SCALE = HD ** -0.5
NCORES = 8
SLAB = N // (T // WT)      # 2048 tokens per (b, it) slab
NWIN = (H // WH) * (W // WW)   # 16 windows per slab
M = WT * WH * WW           # 128 tokens per window
KC = C // 128              # 8 contraction chunks
GW = 4                     # windows per group
NGRP = NWIN // GW
TOKG = M * GW              # 512
NOC = 3 * C // 128         # 24 qkv output chunks

_BUILD_CACHE = {}
bf16 = ml_dtypes.bfloat16


def _split_drain_waits(nc, mybir, cap=1, event_cap=2):
    """This walrus build accepts only one sem wait per TPB instruction
    (Tile's scheduler attaches up to 3).  Move the excess onto
    InstEventSemaphore carriers (which hold 2) inserted right before the
    over-subscribed instruction on the same engine — the engine blocks on the
    carriers first, so semantics are unchanged."""
    for f in nc.m.functions:
        for bb in f.blocks:
            i = 0
            while i < len(bb.instructions):
                ins = bb.instructions[i]
                si = ins.sync_info
                my_cap = (
                    event_cap
                    if type(ins).__name__ == "InstEventSemaphore"
                    else cap
                )
                if si is not None and si.on_wait and len(si.on_wait) > my_cap:
                    waits = list(si.on_wait)
                    si.on_wait = waits[:my_cap]
                    extra = waits[my_cap:]
                    carriers = []
                    while extra:
                        chunk, extra = extra[:event_cap], extra[event_cap:]
                        ev = mybir.InstEventSemaphore(
                            name=f"I-{nc.next_id()}-waitsplit", ins=[], outs=[]
                        )
                        ev.engine = ins.engine
                        ev.sync_info = mybir.SyncInfo(
                            on_wait=list(chunk), on_update=[]
                        )
                        nc.register_instruction(ev)
                        carriers.append(ev)
                    bb.instructions[i:i] = carriers
                    i += len(carriers)
                i += 1


def _build(has_qkvb, has_projb):
    import concourse.bass as bass
    import concourse.tile as tile
    from concourse import mybir
    f32 = mybir.dt.float32
    fpr = mybir.dt.float32r
    bf = mybir.dt.bfloat16

    nc = bass.Bass("TRN2", target_bir_lowering=False, debug=False)
    # host-packed inputs (see _pack_* below)
    xs = nc.dram_tensor("xs", [NGRP, 128, KC, TOKG], bf, kind="ExternalInput")
    wqkv = nc.dram_tensor("wqkv", [16, 128, KC, 128], bf, kind="ExternalInput")
    wv = nc.dram_tensor("wv", [128, KC, C], bf, kind="ExternalInput")
    wproj = nc.dram_tensor("wproj", [128, KC, C], bf, kind="ExternalInput")
    if has_qkvb:
        qkvb = nc.dram_tensor("qkvb", [1, 3 * C], bf, kind="ExternalInput")
    if has_projb:
        projb = nc.dram_tensor("projb", [1, C], bf, kind="ExternalInput")
    outd = nc.dram_tensor("out", [NWIN, M, C], f32, kind="ExternalOutput")

    Exp = mybir.ActivationFunctionType.Exp
    Ln = mybir.ActivationFunctionType.Ln

    with tile.TileContext(nc) as tc:
        with (
            tc.tile_pool(name="wq", bufs=1) as wq_pool,
            tc.tile_pool(name="wp", bufs=1) as wp_pool,
            tc.tile_pool(name="xTp", bufs=1) as xT_pool,
            tc.tile_pool(name="const", bufs=1) as const_pool,
            tc.tile_pool(name="qk", bufs=1) as qk_pool,
            tc.tile_pool(name="v65", bufs=2) as v_pool,
            tc.tile_pool(name="E", bufs=2) as e_pool,
            tc.tile_pool(name="r", bufs=1) as r_pool,
            tc.tile_pool(name="owT", bufs=2) as ow_pool,
            tc.tile_pool(name="o", bufs=2) as o_pool,
            tc.tile_pool(name="psA", bufs=2, space="PSUM") as psA,
            tc.tile_pool(name="psB", bufs=2, space="PSUM") as psB,
            tc.tile_pool(name="psV", bufs=1, space="PSUM") as psV_pool,
        ):
            # oc-major so each chunk's DMA is contiguous on both sides
            # (strided 256B-run destinations were gating startup on the DMA
            # engine); chunks arrive in consumption order QK 0..15 then V.
            wq_sb = wq_pool.tile([128, 16, KC, 128], bf)
            for oc in range(16):
                nc.sync.dma_start(wq_sb[:, oc], wqkv.ap()[oc])
            # V weights in k-major layout so the V matmul rhs slices are
            # contiguous ([128, 512] per bank) instead of 4-chunk strided
            wv_sb = wq_pool.tile([128, KC, C], bf)
            nc.sync.dma_start(wv_sb[:], wv.ap())
            xT = xT_pool.tile([128, NGRP, KC, TOKG], bf)
            # only group 0's x up front; later groups prefetch from inside
            # the previous group so the startup HBM burst is weights + g0
            nc.scalar.dma_start(xT[:, 0], xs.ap()[0])
            wp_sb = wp_pool.tile([128, KC, C], bf)
            nc.sync.dma_start(wp_sb[:], wproj.ap())

            onesf = const_pool.tile([1, TOKG], f32)
            nc.vector.memset(onesf[:], 1.0)
            ones64 = const_pool.tile([1, 64], fpr)
            with nc.allow_low_precision(reason="ones"):
                nc.scalar.copy(ones64[:], onesf[0:1, 0:64])
            ones_col = const_pool.tile([128, GW * NH], f32)
            nc.vector.memset(ones_col[:], 1.0)
            if has_qkvb or has_projb:
                ones_tok = const_pool.tile([1, TOKG], bf)
                with nc.allow_low_precision(reason="ones"):
                    nc.scalar.copy(ones_tok[:], onesf[:])
            if has_qkvb:
                qkvb_sb = const_pool.tile([1, 3 * C], bf)
                nc.sync.dma_start(qkvb_sb[:], qkvb.ap())
            if has_projb:
                projb_sb = const_pool.tile([1, C], bf)
                nc.sync.dma_start(projb_sb[:], projb.ap())

            def finish_hb(owT, psVv, r4, hb):
                """One head-bank of the deferred normalize: broadcast 1/den
                via a K=1 matmul, then two DVE multiplies into owT."""
                Rp = psB.tile([128, 512], f32, tag="psB")
                nc.tensor.matmul(
                    Rp[0:64, :],
                    ones64[:],
                    r4[0:1, hb, :],
                    start=True,
                    stop=True,
                )
                R = r_pool.tile([64, 512], f32, tag="R", bufs=2)
                # alternate engines so consecutive banks' copy+TT chains
                # pipeline instead of queuing on DVE
                if hb % 2 == 0:
                    nc.scalar.copy(R[:], Rp[0:64, :])
                else:
                    nc.vector.tensor_copy(R[:], Rp[0:64, :])
                Rv = R[:].rearrange("p (m2 par t) -> p m2 par t", m2=2, par=2)
                for par in range(2):
                    with nc.allow_low_precision(reason="bf16 owT"):
                        nc.vector.tensor_tensor(
                            owT[64 * par : 64 * (par + 1), 2 * hb : 2 * hb + 2, :],
                            psVv[:, hb, :, par, :],
                            Rv[:, :, par, :],
                            op=mybir.AluOpType.mult,
                        )

            def emit_proj(win, owT):
                """Out-proj matmul chain generator for window win: yields
                after each long matmul so short AV matmuls can interleave
                (hides their LDWEIGHTS under the 512-col proj matmuls)."""
                pps = [psA.tile([128, 512], f32, tag="psA", name=f"pp{i}") for i in range(2)]
                for k in range(KC):
                    for nk in range(2):
                        nc.tensor.matmul(
                            pps[nk][:],
                            owT[:, k, :],
                            wp_sb[:, k, 512 * nk : 512 * (nk + 1)],
                            start=(k == 0),
                            stop=(k == KC - 1 and not has_projb),
                        )
                        yield
                otile = o_pool.tile([128, C], f32)
                for nk in range(2):
                    if has_projb:
                        nc.tensor.matmul(
                            pps[nk][:],
                            ones_tok[0:1, 0:M],
                            projb_sb[0:1, 512 * nk : 512 * (nk + 1)],
                            start=False,
                            stop=True,
                        )
                    # PSUM -> SBUF eviction split across DVE / ACT; DMA
                    # each half as soon as its eviction lands
                    copy = nc.vector.tensor_copy if nk == 0 else nc.scalar.copy
                    copy(otile[:, 512 * nk : 512 * (nk + 1)], pps[nk][:])
                    nc.sync.dma_start(
                        outd.ap()[win][:, 512 * nk : 512 * (nk + 1)],
                        otile[:, 512 * nk : 512 * (nk + 1)],
                    )

            prev = None
            for g in range(NGRP):
                # QKV projection, Q/K head-transposed.  qkT slot h = Q_h
                # (SCALE folded into the host-packed weights), slot 16+h =
                # K_h; head h lives in rows (h%2)*64..+64 of oc-chunk h//2's
                # PSUM block.  Q evictions on DVE, K evictions on ACT.
                # qkT slot h = Q_h (SCALE folded into host weights), slot
                # 16+h = K_h; head h from rows (h%2)*64 of oc-chunk h//2's
                # PSUM block.  All attention operands stay at partition base
                # 0 (mixing base-0/base-64 matmul operands hangs trn2).
                qkT = qk_pool.tile([64, 2 * NH, TOKG], bf)
                for oc in range(16):
                    ps = psA.tile([128, 512], f32, tag="psA")
                    for k in range(KC):
                        nc.tensor.matmul(
                            ps[:],
                            wq_sb[:, oc, k, :],
                            xT[:, g, k, :],
                            start=(k == 0),
                            stop=(k == KC - 1 and not has_qkvb),
                        )
                    if has_qkvb:
                        nc.tensor.matmul(
                            ps[:],
                            qkvb_sb[0:1, 128 * oc : 128 * (oc + 1)],
                            ones_tok[:],
                            start=False,
                            stop=True,
                        )
                    # split each chunk's two evictions across DVE and ACT
                    # so eviction latency never paces the QK matmul chains
                    with nc.allow_low_precision(reason="bf16 qkT"):
                        nc.vector.tensor_copy(qkT[:, 2 * oc, :], ps[0:64, :])
                        nc.scalar.copy(qkT[:, 2 * oc + 1, :], ps[64:128, :])

                v65 = v_pool.tile([128, GW, NH, HD + 1], bf)
                with nc.allow_low_precision(reason="bf16 ones col"):
                    nc.scalar.copy(
                        v65[:, :, :, HD : HD + 1],
                        ones_col[:].rearrange("p (g h) -> p g h", g=GW)[
                            :, :, :, None
                        ],
                    )

                for w in range(GW):
                    last = g == NGRP - 1 and w == GW - 1
                    if w == 0 and g + 1 < NGRP:
                        # prefetch next group's x once startup DMAs are done
                        nc.scalar.dma_start(xT[:, g + 1], xs.ap()[g + 1])
                    # banks 0/1 of prev window's normalize go first: their
                    # recip halves landed mid-window, so the TT chains run
                    # under the V/S block and proj k0 starts unblocked
                    owT_prev = None
                    if prev is not None:
                        _, psV_prev, r4_prev = prev
                        owT_prev = ow_pool.tile([128, KC, M], bf)
                        psVv_prev = psV_prev[0:64, :].rearrange(
                            "p (hb m2 par t) -> p hb m2 par t", hb=4, m2=2, par=2
                        )
                        finish_hb(owT_prev, psVv_prev, r4_prev, 0)
                        finish_hb(owT_prev, psVv_prev, r4_prev, 1)
                        finish_hb(owT_prev, psVv_prev, r4_prev, 2)
                        finish_hb(owT_prev, psVv_prev, r4_prev, 3)
                    # V: token-major, 8 heads per psum bank
                    vps = [psA.tile([128, 512], f32, tag="psA", name=f"vp{i}") for i in range(2)]
                    for nk in range(2):
                        for k in range(KC):
                            nc.tensor.matmul(
                                vps[nk][:],
                                xT[:, g, k, 128 * w : 128 * (w + 1)],
                                wv_sb[:, k, 512 * nk : 512 * (nk + 1)],
                                start=(k == 0),
                                stop=(k == KC - 1 and not has_qkvb),
                            )
                        if has_qkvb:
                            nc.tensor.matmul(
                                vps[nk][:],
                                ones_tok[0:1, 0:M],
                                qkvb_sb[0:1, 2 * C + 512 * nk : 2 * C + 512 * (nk + 1)],
                                start=False,
                                stop=True,
                            )
                        with nc.allow_low_precision(reason="bf16 v65"):
                            nc.vector.tensor_copy(
                                v65[:, w, 8 * nk : 8 * nk + 8, 0:HD],
                                vps[nk][:].rearrange("p (h e) -> p h e", e=HD),
                            )

                    # S^T per head (bf16, ap=128), exp per 4-head bank
                    E = e_pool.tile([128, NH * M], bf)
                    for hb in range(4):
                        psS = psB.tile([128, 512], f32, tag="psB")
                        for m in range(4):
                            h = 4 * hb + m
                            nc.tensor.matmul(
                                psS[:, 128 * m : 128 * (m + 1)],
                                qkT[:, NH + h, 128 * w : 128 * (w + 1)],
                                qkT[:, h, 128 * w : 128 * (w + 1)],
                                start=True,
                                stop=True,
                            )
                        with nc.allow_low_precision(reason="bf16 E"):
                            nc.scalar.activation(
                                E[:, 512 * hb : 512 * (hb + 1)], psS[:], Exp
                            )

                    # previous window's remaining banks + out-proj (PE
                    # side covered by the V/S work above)
                    proj_gen = None
                    if prev is not None:
                        proj_gen = emit_proj(prev[0], owT_prev)

                    # A.V interleaved with prev's out-proj long matmuls
                    psV = psV_pool.tile([128, 4 * 512], f32)
                    r4 = r_pool.tile([1, 4, 512], fpr, tag="r4", bufs=2)
                    for h in range(NH):
                        if proj_gen is not None and not last:
                            # (last window: AV runs first so its recip
                            # chains start early; prev proj drains after)
                            next(proj_gen, None)
                        nc.tensor.matmul(
                            psV[0:65, 128 * h : 128 * (h + 1)],
                            v65[:, w, h, :],
                            E[:, 128 * h : 128 * (h + 1)],
                            start=True,
                            stop=True,
                        )
                        if last and h % 4 == 3:
                            # last window: per-bank recip so the tail
                            # normalize/proj can pipeline per head-bank
                            hb = h // 4
                            L = r_pool.tile([1, 512], f32, tag="L", bufs=2)
                            nc.scalar.activation(
                                L[:], psV[64:65, 512 * hb : 512 * (hb + 1)], Ln
                            )
                            with nc.allow_low_precision(reason="f32r recip"):
                                nc.scalar.activation(
                                    r4[0:1, hb, :], L[:], Exp, scale=-1.0
                                )
                        elif h % 8 == 7:
                            # half-window recip: bank pair's chain starts as
                            # soon as its 8 AV matmuls land, so most of the
                            # recip latency hides under the proj/AV block
                            hh = h // 8
                            L = r_pool.tile([1, 1024], f32, tag="Lh", bufs=2)
                            nc.scalar.activation(
                                L[:], psV[64:65, 1024 * hh : 1024 * (hh + 1)], Ln
                            )
                            with nc.allow_low_precision(reason="f32r recip"):
                                nc.scalar.activation(
                                    r4[0:1, 2 * hh : 2 * hh + 2, :].rearrange(
                                        "o hb t -> o (hb t)"
                                    ),
                                    L[:],
                                    Exp,
                                    scale=-1.0,
                                )
                    if proj_gen is not None:
                        for _ in proj_gen:
                            pass
                    prev = (g * GW + w, psV, r4)

            # tail: last window's normalize + out-proj, pipelined per
            # head-bank (finish_hb(hb) unblocks proj k-chunks 2hb, 2hb+1)
            _, psV_prev, r4_prev = prev
            owT_prev = ow_pool.tile([128, KC, M], bf)
            psVv_prev = psV_prev[0:64, :].rearrange(
                "p (hb m2 par t) -> p hb m2 par t", hb=4, m2=2, par=2
            )
            pps = [psA.tile([128, 512], f32, tag="psA", name=f"tpp{i}") for i in range(2)]
            for hb in range(4):
                finish_hb(owT_prev, psVv_prev, r4_prev, hb)
                for k in (2 * hb, 2 * hb + 1):
                    for nk in range(2):
                        nc.tensor.matmul(
                            pps[nk][:],
                            owT_prev[:, k, :],
                            wp_sb[:, k, 512 * nk : 512 * (nk + 1)],
                            start=(k == 0),
                            stop=(k == KC - 1 and not has_projb),
                        )
            otile = o_pool.tile([128, C], f32)
            for nk in range(2):
                if has_projb:
                    nc.tensor.matmul(
                        pps[nk][:],
                        ones_tok[0:1, 0:M],
                        projb_sb[0:1, 512 * nk : 512 * (nk + 1)],
                        start=False,
                        stop=True,
                    )
                copy = nc.vector.tensor_copy if nk == 0 else nc.scalar.copy
                copy(otile[:, 512 * nk : 512 * (nk + 1)], pps[nk][:])
                nc.sync.dma_start(
                    outd.ap()[prev[0]][:, 512 * nk : 512 * (nk + 1)],
                    otile[:, 512 * nk : 512 * (nk + 1)],
                )

    _split_drain_waits(nc, mybir)
    return nc


def _get_nc(has_qkvb, has_projb):
    key = (has_qkvb, has_projb)
    if key not in _BUILD_CACHE:
        _BUILD_CACHE[key] = _build(has_qkvb, has_projb)
    return _BUILD_CACHE[key]


def _pack_weights(qkv_w, proj_w):
    # SCALE is folded into the Q weights so the kernel's qkT evictions are
    # plain copies.  wqkv: [oc, p, k, j] from qkv_w.T[c, o]; c = k*128+p,
    # o = oc*128+j
    qkv_w = qkv_w.copy()
    qkv_w[:C] *= SCALE
    wqkT = qkv_w.T.astype(bf16)                      # [C, 3C]
    wq = np.ascontiguousarray(
        wqkT[:, : 2 * C].reshape(KC, 128, 16, 128).transpose(2, 1, 0, 3)
    )
    wv = np.ascontiguousarray(
        wqkT[:, 2 * C :].reshape(KC, 128, C).transpose(1, 0, 2)
    )
    wp = np.ascontiguousarray(
        proj_w.T.astype(bf16).reshape(KC, 128, C).transpose(1, 0, 2)
    )
    return wq, wv, wp


def _pack_x_slab(xslab):
    # xslab [2048, C] tokens in (tt, ih, hh, iw, ww) order ->
    # [NGRP, 128 c-part, KC, TOKG] with windows (ih, iw) grouped by 4,
    # intra-window token (tt, hh, ww)
    xw = (
        xslab.reshape(WT, 4, WH, 4, WW, C)
        .transpose(1, 3, 0, 2, 4, 5)
        .reshape(NWIN, M, C)
        .astype(bf16)
    )
    # [win, tok, c] -> [g, p, k, w_in_g*128+tok]; c = k*128+p
    xt = (
        xw.reshape(NGRP, GW, M, KC, 128)
        .transpose(0, 4, 3, 1, 2)
        .reshape(NGRP, 128, KC, TOKG)
    )
    return np.ascontiguousarray(xt)


def _unpack_out(owin):
    # [NWIN(ih,iw), M(tt,hh,ww), C] -> [2048(tt,ih,hh,iw,ww), C]
    return (
        owin.reshape(4, 4, WT, WH, WW, C)
        .transpose(2, 0, 3, 1, 4, 5)
        .reshape(SLAB, C)
    )


def prepare_in_maps(x, qkv_w, qkv_b, proj_w, proj_b):
    has_qkvb = bool(np.any(qkv_b))
    has_projb = bool(np.any(proj_b))
    wq, wv, wp = _pack_weights(qkv_w, proj_w)
    in_maps = []
    for core in range(NCORES):
        b, it = divmod(core, T // WT)
        im = {
            "xs": _pack_x_slab(x[b, it * SLAB : (it + 1) * SLAB, :]),
            "wqkv": wq,
            "wv": wv,
            "wproj": wp,
        }
        if has_qkvb:
            qb = qkv_b.copy()
            qb[:C] *= SCALE
            im["qkvb"] = qb.reshape(1, 3 * C).astype(bf16)
        if has_projb:
            im["projb"] = proj_b.reshape(1, C).astype(bf16)
        in_maps.append(im)
    return in_maps, has_qkvb, has_projb


def kernel(x, qkv_w, qkv_b, proj_w, proj_b, t, h, w, **_unused):
    from concourse.bass_utils import run_bass_kernel_spmd

    x = np.asarray(x, dtype=np.float32)
    qkv_w = np.asarray(qkv_w, dtype=np.float32)
    qkv_b = np.asarray(qkv_b, dtype=np.float32)
    proj_w = np.asarray(proj_w, dtype=np.float32)
    proj_b = np.asarray(proj_b, dtype=np.float32)
    assert x.shape == (B, N, C), x.shape
    assert int(t) == T and int(h) == H and int(w) == W

    in_maps, has_qkvb, has_projb = prepare_in_maps(
        x, qkv_w, qkv_b, proj_w, proj_b
    )
    nc = _get_nc(has_qkvb, has_projb)
    res = run_bass_kernel_spmd(nc, in_maps, core_ids=list(range(NCORES)))

    y = np.empty((B, N, C), dtype=np.float32)
    for core in range(NCORES):
        b, it = divmod(core, T // WT)
        y[b, it * SLAB : (it + 1) * SLAB, :] = _unpack_out(
            res.results[core]["out"]
        )
    return y


# revision 32
# speedup vs baseline: 1.0205x; 1.0205x over previous
"""Trainium2 Bass kernel for windowed (block-diagonal) multi-head video attention.

Problem: x:[2,8192,1024] -> qkv proj -> 3D-window (2,8,8) attention over a
(8,32,32) token grid, 16 heads x 64 dim -> out proj -> [2,8192,1024].

Sharding: 8 cores, data-parallel over (batch, t-window-group); the slab
x[b, it*2048:(it+1)*2048, :] holds the 16 independent (h,w)-windows with
t in {2it, 2it+1}.

This version runs the whole datapath in bf16 on the PE (1 cyc/col at any
ap size, vs f32r's 4 cyc/col below ap=256, so the ap=128 attention matmuls
are 4x faster) and moves all layout work to the host:
  - x is pre-transposed and window-gathered on the host into
    [group, 128 c-part, KC, 512 tok] so the kernel needs no PE transposes
    and no strided gather DMAs; qkv/proj weights are pre-chunked so each
    oc-chunk is one contiguous DMA that arrives in consumption order.
  - windows processed in groups of GW=4 (tok dim 512 = full PSUM bank).
  - attention: S^T = K_h Q_h^T per head (bf16), exp on ACT, A.V matmul with
    a per-head ones column producing the softmax denominator for free; the
    16 head-blocks of a window share one 4-bank PSUM tile and the recip
    (Ln + exp(-x) on ACT) runs per half-window as soon as each 8-head
    denominator stretch lands; 1/den is partition-broadcast via a K=1
    matmul and applied on DVE (R copies alternate ACT/DVE).
  - window finish (recip-broadcast, normalize, out-proj) is software-
    pipelined one window behind S/AV: all four normalize chains are emitted
    at the top of the next window (their recips completed mid-window), the
    V/S block covers them, and the out-proj interleaves with the next A.V
    so the PE never waits on the ACT/DVE chains.
Output is written window-major and un-permuted on the host.
"""

import sys

for _p in ("/opt/trn_rl_repo",):
    if _p not in sys.path:
        sys.path.insert(0, _p)

import numpy as np
import ml_dtypes

B, T, H, W = 2, 8, 32, 32
C, NH, HD = 1024, 16, 64
WT, WH, WW = 2, 8, 8
N = T * H * W              # 8192 tokens
SCALE = HD ** -0.5
NCORES = 8
SLAB = N // (T // WT)      # 2048 tokens per (b, it) slab
NWIN = (H // WH) * (W // WW)   # 16 windows per slab
M = WT * WH * WW           # 128 tokens per window
KC = C // 128              # 8 contraction chunks
GW = 4                     # windows per group
NGRP = NWIN // GW
TOKG = M * GW              # 512
NOC = 3 * C // 128         # 24 qkv output chunks

_BUILD_CACHE = {}
bf16 = ml_dtypes.bfloat16


def _split_drain_waits(nc, mybir, cap=1, event_cap=2):
    """This walrus build accepts only one sem wait per TPB instruction
    (Tile's scheduler attaches up to 3).  Move the excess onto
    InstEventSemaphore carriers (which hold 2) inserted right before the
    over-subscribed instruction on the same engine — the engine blocks on the
    carriers first, so semantics are unchanged."""
    for f in nc.m.functions:
        for bb in f.blocks:
            i = 0
            while i < len(bb.instructions):
                ins = bb.instructions[i]
                si = ins.sync_info
                my_cap = (
                    event_cap
                    if type(ins).__name__ == "InstEventSemaphore"
                    else cap
                )
                if si is not None and si.on_wait and len(si.on_wait) > my_cap:
                    waits = list(si.on_wait)
                    si.on_wait = waits[:my_cap]
                    extra = waits[my_cap:]
                    carriers = []
                    while extra:
                        chunk, extra = extra[:event_cap], extra[event_cap:]
                        ev = mybir.InstEventSemaphore(
                            name=f"I-{nc.next_id()}-waitsplit", ins=[], outs=[]
                        )
                        ev.engine = ins.engine
                        ev.sync_info = mybir.SyncInfo(
                            on_wait=list(chunk), on_update=[]
                        )
                        nc.register_instruction(ev)
                        carriers.append(ev)
                    bb.instructions[i:i] = carriers
                    i += len(carriers)
                i += 1


def _build(has_qkvb, has_projb):
    import concourse.bass as bass
    import concourse.tile as tile
    from concourse import mybir
    f32 = mybir.dt.float32
    fpr = mybir.dt.float32r
    bf = mybir.dt.bfloat16

    nc = bass.Bass("TRN2", target_bir_lowering=False, debug=False)
    # host-packed inputs (see _pack_* below)
    xs = nc.dram_tensor("xs", [NGRP, 128, KC, TOKG], bf, kind="ExternalInput")
    wqkv = nc.dram_tensor("wqkv", [16, 128, KC, 128], bf, kind="ExternalInput")
    wv = nc.dram_tensor("wv", [128, KC, C], bf, kind="ExternalInput")
    wproj = nc.dram_tensor("wproj", [128, KC, C], bf, kind="ExternalInput")
    if has_qkvb:
        qkvb = nc.dram_tensor("qkvb", [1, 3 * C], bf, kind="ExternalInput")
    if has_projb:
        projb = nc.dram_tensor("projb", [1, C], bf, kind="ExternalInput")
    outd = nc.dram_tensor("out", [NWIN, M, C], f32, kind="ExternalOutput")

    Exp = mybir.ActivationFunctionType.Exp
    Ln = mybir.ActivationFunctionType.Ln

    with tile.TileContext(nc) as tc:
        with (
            tc.tile_pool(name="wq", bufs=1) as wq_pool,
            tc.tile_pool(name="wp", bufs=1) as wp_pool,
            tc.tile_pool(name="xTp", bufs=1) as xT_pool,
            tc.tile_pool(name="const", bufs=1) as const_pool,
            tc.tile_pool(name="qk", bufs=1) as qk_pool,
            tc.tile_pool(name="v65", bufs=2) as v_pool,
            tc.tile_pool(name="E", bufs=2) as e_pool,
            tc.tile_pool(name="r", bufs=1) as r_pool,
            tc.tile_pool(name="owT", bufs=2) as ow_pool,
            tc.tile_pool(name="o", bufs=2) as o_pool,
            tc.tile_pool(name="psA", bufs=2, space="PSUM") as psA,
            tc.tile_pool(name="psB", bufs=2, space="PSUM") as psB,
            tc.tile_pool(name="psV", bufs=1, space="PSUM") as psV_pool,
        ):
            # oc-major so each chunk's DMA is contiguous on both sides
            # (strided 256B-run destinations were gating startup on the DMA
            # engine); chunks arrive in consumption order QK 0..15 then V.
            wq_sb = wq_pool.tile([128, 16, KC, 128], bf)
            for oc in range(16):
                nc.sync.dma_start(wq_sb[:, oc], wqkv.ap()[oc])
            # V weights in k-major layout so the V matmul rhs slices are
            # contiguous ([128, 512] per bank) instead of 4-chunk strided
            wv_sb = wq_pool.tile([128, KC, C], bf)
            nc.sync.dma_start(wv_sb[:], wv.ap())
            xT = xT_pool.tile([128, NGRP, KC, TOKG], bf)
            # only group 0's x up front; later groups prefetch from inside
            # the previous group so the startup HBM burst is weights + g0
            nc.scalar.dma_start(xT[:, 0], xs.ap()[0])
            wp_sb = wp_pool.tile([128, KC, C], bf)
            nc.sync.dma_start(wp_sb[:], wproj.ap())

            onesf = const_pool.tile([1, TOKG], f32)
            nc.vector.memset(onesf[:], 1.0)
            ones64 = const_pool.tile([1, 64], fpr)
            with nc.allow_low_precision(reason="ones"):
                nc.scalar.copy(ones64[:], onesf[0:1, 0:64])
            ones_col = const_pool.tile([128, GW * NH], f32)
            nc.vector.memset(ones_col[:], 1.0)
            if has_qkvb or has_projb:
                ones_tok = const_pool.tile([1, TOKG], bf)
                with nc.allow_low_precision(reason="ones"):
                    nc.scalar.copy(ones_tok[:], onesf[:])
            if has_qkvb:
                qkvb_sb = const_pool.tile([1, 3 * C], bf)
                nc.sync.dma_start(qkvb_sb[:], qkvb.ap())
            if has_projb:
                projb_sb = const_pool.tile([1, C], bf)
                nc.sync.dma_start(projb_sb[:], projb.ap())

            def finish_hb(owT, psVv, r4, hb):
                """One head-bank of the deferred normalize: broadcast 1/den
                via a K=1 matmul, then two DVE multiplies into owT."""
                Rp = psB.tile([128, 512], f32, tag="psB")
                nc.tensor.matmul(
                    Rp[0:64, :],
                    ones64[:],
                    r4[0:1, hb, :],
                    start=True,
                    stop=True,
                )
                R = r_pool.tile([64, 512], f32, tag="R", bufs=2)
                # alternate engines so consecutive banks' copy+TT chains
                # pipeline instead of queuing on DVE
                if hb % 2 == 0:
                    nc.scalar.copy(R[:], Rp[0:64, :])
                else:
                    nc.vector.tensor_copy(R[:], Rp[0:64, :])
                Rv = R[:].rearrange("p (m2 par t) -> p m2 par t", m2=2, par=2)
                for par in range(2):
                    with nc.allow_low_precision(reason="bf16 owT"):
                        nc.vector.tensor_tensor(
                            owT[64 * par : 64 * (par + 1), 2 * hb : 2 * hb + 2, :],
                            psVv[:, hb, :, par, :],
                            Rv[:, :, par, :],
                            op=mybir.AluOpType.mult,
                        )

            def emit_proj(win, owT):
                """Out-proj matmul chain generator for window win: yields
                after each long matmul so short AV matmuls can interleave
                (hides their LDWEIGHTS under the 512-col proj matmuls)."""
                pps = [psA.tile([128, 512], f32, tag="psA", name=f"pp{i}") for i in range(2)]
                for k in range(KC):
                    for nk in range(2):
                        nc.tensor.matmul(
                            pps[nk][:],
                            owT[:, k, :],
                            wp_sb[:, k, 512 * nk : 512 * (nk + 1)],
                            start=(k == 0),
                            stop=(k == KC - 1 and not has_projb),
                        )
                        yield
                otile = o_pool.tile([128, C], f32)
                for nk in range(2):
                    if has_projb:
                        nc.tensor.matmul(
                            pps[nk][:],
                            ones_tok[0:1, 0:M],
                            projb_sb[0:1, 512 * nk : 512 * (nk + 1)],
                            start=False,
                            stop=True,
                        )
                    # PSUM -> SBUF eviction split across DVE / ACT; DMA
                    # each half as soon as its eviction lands
                    copy = nc.vector.tensor_copy if nk == 0 else nc.scalar.copy
                    copy(otile[:, 512 * nk : 512 * (nk + 1)], pps[nk][:])
                    nc.sync.dma_start(
                        outd.ap()[win][:, 512 * nk : 512 * (nk + 1)],
                        otile[:, 512 * nk : 512 * (nk + 1)],
                    )

            prev = None
            for g in range(NGRP):
                # QKV projection, Q/K head-transposed.  qkT slot h = Q_h
                # (SCALE folded into the host-packed weights), slot 16+h =
                # K_h; head h lives in rows (h%2)*64..+64 of oc-chunk h//2's
                # PSUM block.  Q evictions on DVE, K evictions on ACT.
                # qkT slot h = Q_h (SCALE folded into host weights), slot
                # 16+h = K_h; head h from rows (h%2)*64 of oc-chunk h//2's
                # PSUM block.  All attention operands stay at partition base
                # 0 (mixing base-0/base-64 matmul operands hangs trn2).
                qkT = qk_pool.tile([64, 2 * NH, TOKG], bf)
                for oc in range(16):
                    ps = psA.tile([128, 512], f32, tag="psA")
                    for k in range(KC):
                        nc.tensor.matmul(
                            ps[:],
                            wq_sb[:, oc, k, :],
                            xT[:, g, k, :],
                            start=(k == 0),
                            stop=(k == KC - 1 and not has_qkvb),
                        )
                    if has_qkvb:
                        nc.tensor.matmul(
                            ps[:],
                            qkvb_sb[0:1, 128 * oc : 128 * (oc + 1)],
                            ones_tok[:],
                            start=False,
                            stop=True,
                        )
                    copy = nc.vector.tensor_copy if oc < 8 else nc.scalar.copy
                    with nc.allow_low_precision(reason="bf16 qkT"):
                        copy(qkT[:, 2 * oc, :], ps[0:64, :])
                        copy(qkT[:, 2 * oc + 1, :], ps[64:128, :])

                v65 = v_pool.tile([128, GW, NH, HD + 1], bf)
                with nc.allow_low_precision(reason="bf16 ones col"):
                    nc.scalar.copy(
                        v65[:, :, :, HD : HD + 1],
                        ones_col[:].rearrange("p (g h) -> p g h", g=GW)[
                            :, :, :, None
                        ],
                    )

                for w in range(GW):
                    last = g == NGRP - 1 and w == GW - 1
                    if w == 0 and g + 1 < NGRP:
                        # prefetch next group's x once startup DMAs are done
                        nc.scalar.dma_start(xT[:, g + 1], xs.ap()[g + 1])
                    # banks 0/1 of prev window's normalize go first: their
                    # recip halves landed mid-window, so the TT chains run
                    # under the V/S block and proj k0 starts unblocked
                    owT_prev = None
                    if prev is not None:
                        _, psV_prev, r4_prev = prev
                        owT_prev = ow_pool.tile([128, KC, M], bf)
                        psVv_prev = psV_prev[0:64, :].rearrange(
                            "p (hb m2 par t) -> p hb m2 par t", hb=4, m2=2, par=2
                        )
                        finish_hb(owT_prev, psVv_prev, r4_prev, 0)
                        finish_hb(owT_prev, psVv_prev, r4_prev, 1)
                        finish_hb(owT_prev, psVv_prev, r4_prev, 2)
                        finish_hb(owT_prev, psVv_prev, r4_prev, 3)
                    # V: token-major, 8 heads per psum bank
                    vps = [psA.tile([128, 512], f32, tag="psA", name=f"vp{i}") for i in range(2)]
                    for nk in range(2):
                        for k in range(KC):
                            nc.tensor.matmul(
                                vps[nk][:],
                                xT[:, g, k, 128 * w : 128 * (w + 1)],
                                wv_sb[:, k, 512 * nk : 512 * (nk + 1)],
                                start=(k == 0),
                                stop=(k == KC - 1 and not has_qkvb),
                            )
                        if has_qkvb:
                            nc.tensor.matmul(
                                vps[nk][:],
                                ones_tok[0:1, 0:M],
                                qkvb_sb[0:1, 2 * C + 512 * nk : 2 * C + 512 * (nk + 1)],
                                start=False,
                                stop=True,
                            )
                        with nc.allow_low_precision(reason="bf16 v65"):
                            nc.vector.tensor_copy(
                                v65[:, w, 8 * nk : 8 * nk + 8, 0:HD],
                                vps[nk][:].rearrange("p (h e) -> p h e", e=HD),
                            )

                    # S^T per head (bf16, ap=128), exp per 4-head bank
                    E = e_pool.tile([128, NH * M], bf)
                    for hb in range(4):
                        psS = psB.tile([128, 512], f32, tag="psB")
                        for m in range(4):
                            h = 4 * hb + m
                            nc.tensor.matmul(
                                psS[:, 128 * m : 128 * (m + 1)],
                                qkT[:, NH + h, 128 * w : 128 * (w + 1)],
                                qkT[:, h, 128 * w : 128 * (w + 1)],
                                start=True,
                                stop=True,
                            )
                        with nc.allow_low_precision(reason="bf16 E"):
                            nc.scalar.activation(
                                E[:, 512 * hb : 512 * (hb + 1)], psS[:], Exp
                            )

                    # previous window's remaining banks + out-proj (PE
                    # side covered by the V/S work above)
                    proj_gen = None
                    if prev is not None:
                        proj_gen = emit_proj(prev[0], owT_prev)

                    # A.V interleaved with prev's out-proj long matmuls
                    psV = psV_pool.tile([128, 4 * 512], f32)
                    r4 = r_pool.tile([1, 4, 512], fpr, tag="r4", bufs=2)
                    for h in range(NH):
                        if proj_gen is not None:
                            next(proj_gen, None)
                        nc.tensor.matmul(
                            psV[0:65, 128 * h : 128 * (h + 1)],
                            v65[:, w, h, :],
                            E[:, 128 * h : 128 * (h + 1)],
                            start=True,
                            stop=True,
                        )
                        if last and h % 4 == 3:
                            # last window: per-bank recip so the tail
                            # normalize/proj can pipeline per head-bank
                            hb = h // 4
                            L = r_pool.tile([1, 512], f32, tag="L", bufs=2)
                            nc.scalar.activation(
                                L[:], psV[64:65, 512 * hb : 512 * (hb + 1)], Ln
                            )
                            with nc.allow_low_precision(reason="f32r recip"):
                                nc.scalar.activation(
                                    r4[0:1, hb, :], L[:], Exp, scale=-1.0
                                )
                        elif h % 8 == 7:
                            # half-window recip: bank pair's chain starts as
                            # soon as its 8 AV matmuls land, so most of the
                            # recip latency hides under the proj/AV block
                            hh = h // 8
                            L = r_pool.tile([1, 1024], f32, tag="Lh", bufs=2)
                            nc.scalar.activation(
                                L[:], psV[64:65, 1024 * hh : 1024 * (hh + 1)], Ln
                            )
                            with nc.allow_low_precision(reason="f32r recip"):
                                nc.scalar.activation(
                                    r4[0:1, 2 * hh : 2 * hh + 2, :].rearrange(
                                        "o hb t -> o (hb t)"
                                    ),
                                    L[:],
                                    Exp,
                                    scale=-1.0,
                                )
                    if proj_gen is not None:
                        for _ in proj_gen:
                            pass
                    prev = (g * GW + w, psV, r4)

            # tail: last window's normalize + out-proj, pipelined per
            # head-bank (finish_hb(hb) unblocks proj k-chunks 2hb, 2hb+1)
            _, psV_prev, r4_prev = prev
            owT_prev = ow_pool.tile([128, KC, M], bf)
            psVv_prev = psV_prev[0:64, :].rearrange(
                "p (hb m2 par t) -> p hb m2 par t", hb=4, m2=2, par=2
            )
            pps = [psA.tile([128, 512], f32, tag="psA", name=f"tpp{i}") for i in range(2)]
            for hb in range(4):
                finish_hb(owT_prev, psVv_prev, r4_prev, hb)
                for k in (2 * hb, 2 * hb + 1):
                    for nk in range(2):
                        nc.tensor.matmul(
                            pps[nk][:],
                            owT_prev[:, k, :],
                            wp_sb[:, k, 512 * nk : 512 * (nk + 1)],
                            start=(k == 0),
                            stop=(k == KC - 1 and not has_projb),
                        )
            otile = o_pool.tile([128, C], f32)
            for nk in range(2):
                if has_projb:
                    nc.tensor.matmul(
                        pps[nk][:],
                        ones_tok[0:1, 0:M],
                        projb_sb[0:1, 512 * nk : 512 * (nk + 1)],
                        start=False,
                        stop=True,
                    )
                copy = nc.vector.tensor_copy if nk == 0 else nc.scalar.copy
                copy(otile[:, 512 * nk : 512 * (nk + 1)], pps[nk][:])
                nc.sync.dma_start(
                    outd.ap()[prev[0]][:, 512 * nk : 512 * (nk + 1)],
                    otile[:, 512 * nk : 512 * (nk + 1)],
                )

    _split_drain_waits(nc, mybir)
    return nc


def _get_nc(has_qkvb, has_projb):
    key = (has_qkvb, has_projb)
    if key not in _BUILD_CACHE:
        _BUILD_CACHE[key] = _build(has_qkvb, has_projb)
    return _BUILD_CACHE[key]


def _pack_weights(qkv_w, proj_w):
    # SCALE is folded into the Q weights so the kernel's qkT evictions are
    # plain copies.  wqkv: [oc, p, k, j] from qkv_w.T[c, o]; c = k*128+p,
    # o = oc*128+j
    qkv_w = qkv_w.copy()
    qkv_w[:C] *= SCALE
    wqkT = qkv_w.T.astype(bf16)                      # [C, 3C]
    wq = np.ascontiguousarray(
        wqkT[:, : 2 * C].reshape(KC, 128, 16, 128).transpose(2, 1, 0, 3)
    )
    wv = np.ascontiguousarray(
        wqkT[:, 2 * C :].reshape(KC, 128, C).transpose(1, 0, 2)
    )
    wp = np.ascontiguousarray(
        proj_w.T.astype(bf16).reshape(KC, 128, C).transpose(1, 0, 2)
    )
    return wq, wv, wp


def _pack_x_slab(xslab):
    # xslab [2048, C] tokens in (tt, ih, hh, iw, ww) order ->
    # [NGRP, 128 c-part, KC, TOKG] with windows (ih, iw) grouped by 4,
    # intra-window token (tt, hh, ww)
    xw = (
        xslab.reshape(WT, 4, WH, 4, WW, C)
        .transpose(1, 3, 0, 2, 4, 5)
        .reshape(NWIN, M, C)
        .astype(bf16)
    )
    # [win, tok, c] -> [g, p, k, w_in_g*128+tok]; c = k*128+p
    xt = (
        xw.reshape(NGRP, GW, M, KC, 128)
        .transpose(0, 4, 3, 1, 2)
        .reshape(NGRP, 128, KC, TOKG)
    )
    return np.ascontiguousarray(xt)


def _unpack_out(owin):
    # [NWIN(ih,iw), M(tt,hh,ww), C] -> [2048(tt,ih,hh,iw,ww), C]
    return (
        owin.reshape(4, 4, WT, WH, WW, C)
        .transpose(2, 0, 3, 1, 4, 5)
        .reshape(SLAB, C)
    )


def prepare_in_maps(x, qkv_w, qkv_b, proj_w, proj_b):
    has_qkvb = bool(np.any(qkv_b))
    has_projb = bool(np.any(proj_b))
    wq, wv, wp = _pack_weights(qkv_w, proj_w)
    in_maps = []
    for core in range(NCORES):
        b, it = divmod(core, T // WT)
        im = {
            "xs": _pack_x_slab(x[b, it * SLAB : (it + 1) * SLAB, :]),
            "wqkv": wq,
            "wv": wv,
            "wproj": wp,
        }
        if has_qkvb:
            qb = qkv_b.copy()
            qb[:C] *= SCALE
            im["qkvb"] = qb.reshape(1, 3 * C).astype(bf16)
        if has_projb:
            im["projb"] = proj_b.reshape(1, C).astype(bf16)
        in_maps.append(im)
    return in_maps, has_qkvb, has_projb


def kernel(x, qkv_w, qkv_b, proj_w, proj_b, t, h, w, **_unused):
    from concourse.bass_utils import run_bass_kernel_spmd

    x = np.asarray(x, dtype=np.float32)
    qkv_w = np.asarray(qkv_w, dtype=np.float32)
    qkv_b = np.asarray(qkv_b, dtype=np.float32)
    proj_w = np.asarray(proj_w, dtype=np.float32)
    proj_b = np.asarray(proj_b, dtype=np.float32)
    assert x.shape == (B, N, C), x.shape
    assert int(t) == T and int(h) == H and int(w) == W

    in_maps, has_qkvb, has_projb = prepare_in_maps(
        x, qkv_w, qkv_b, proj_w, proj_b
    )
    nc = _get_nc(has_qkvb, has_projb)
    res = run_bass_kernel_spmd(nc, in_maps, core_ids=list(range(NCORES)))

    y = np.empty((B, N, C), dtype=np.float32)
    for core in range(NCORES):
        b, it = divmod(core, T // WT)
        y[b, it * SLAB : (it + 1) * SLAB, :] = _unpack_out(
            res.results[core]["out"]
        )
    return y
